# revision 8
# baseline (speedup 1.0000x reference)
"""Bass/Trainium2 kernel for batched GNN message passing:
    out[b, d, n] = sum_m adj[b, n, m] * x[b, d, m]
B=2, D=3072, N=8192, fp32.

Sharding: 8 cores, core c -> (b = c//4, n-quarter = c%4). Each core computes
C[d, n_quarter] = X[b] @ A[b, n_quarter, :].T  with D=3072, NC=2048, M=8192.
Zero collectives; host slices inputs and concatenates outputs.

v2: fp16 operands, host-side transpose (no on-device transposes at all).
Host feeds xT pre-tiled as [kq, db, p, kc, dd] and adjT as plain [M, NC];
per K-pass the kernel streams back-to-back fp16 matmuls (1 cyc/row) into
4 PSUM banks per d-block, alternating bank sets so eviction overlaps
compute. Partial sums across the kq K-passes round-trip DRAM in fp16.
"""

import sys
from contextlib import ExitStack

import numpy as np

sys.path.insert(0, "/opt/trn_rl_repo")

B = 2
D = 3072
N = 8192
NCORES = 8
NSPLIT = 4  # n-quarters per batch sample
NC = N // NSPLIT  # 2048 columns of out per core
KQ = 4  # K-passes
P = 128


def build_program(d=D, ncols=NC, m=N, kq=KQ, nbw=512):
    """Build the per-core Bass program. Returns compiled nc."""
    import concourse.mybir as mybir
    import concourse.tile as tile
    from concourse import bacc

    f32 = mybir.dt.float32
    f16 = mybir.dt.float16

    mq = m // kq          # contraction elems per K-pass
    kc_n = mq // P        # 128-chunks per K-pass
    ndb = d // P          # d-blocks
    nnb = ncols // nbw    # psum banks swept per d-block

    nc = bacc.Bacc(None, target_bir_lowering=False, debug=False)

    # xT tiled on host: x_ext[kqi, db, p, kc, dd] = x[db*P+dd, kqi*mq+kc*P+p]
    x_ext = nc.dram_tensor("x", [kq, ndb, P, kc_n, P], f16, kind="ExternalInput")
    # adjT plain: a_ext[m, n]
    a_ext = nc.dram_tensor("adj", [m, ncols], f16, kind="ExternalInput")
    out_ext = nc.dram_tensor("out", [d, ncols], f32, kind="ExternalOutput")

    with tile.TileContext(nc) as tc, ExitStack() as ctx:
        dram = ctx.enter_context(tc.tile_pool(name="dram", bufs=1, space="DRAM"))
        c_accum = None
        if kq > 1:
            c_accum = dram.tile([ndb, P, ncols], f16, name="c_accum")

        panel_pool = ctx.enter_context(tc.tile_pool(name="panel", bufs=2))
        xt_pool = ctx.enter_context(tc.tile_pool(name="xt", bufs=4))
        stag_pool = ctx.enter_context(tc.tile_pool(name="stag", bufs=3))
        out_pool = ctx.enter_context(tc.tile_pool(name="outp", bufs=2))
        cprev_pool = ctx.enter_context(tc.tile_pool(name="cprev", bufs=3))
        acc_psum = ctx.enter_context(tc.tile_pool(name="accp", bufs=8, space="PSUM"))

        for kqi in range(kq):
            mlo = kqi * mq
            # ---- load adjT panel [P, kc_n, ncols] for this K-pass ----
            panel = panel_pool.tile([P, kc_n, ncols], f16, tag="panel")
            if kqi == 0:
                # First panel gates kernel start: split each kc row into two
                # half-width chunks on separate HWDGE engines so chunks land in
                # kc order across many DMA queues and matmuls can trickle-start
                # at DMA-arrival rate instead of waiting for the whole panel.
                csz = ncols // 2
                for kc in range(kc_n):
                    for q, eng in enumerate((nc.sync, nc.scalar)):
                        eng.dma_start(
                            out=panel[:, kc, q * csz : (q + 1) * csz],
                            in_=a_ext[
                                mlo + kc * P : mlo + (kc + 1) * P,
                                q * csz : (q + 1) * csz,
                            ],
                        )
            else:
                for kc in range(kc_n):
                    nc.sync.dma_start(
                        out=panel[:, kc, :],
                        in_=a_ext[mlo + kc * P : mlo + (kc + 1) * P, :],
                    )

            # ---- d-block loop ----
            for db in range(ndb):
                xt = xt_pool.tile([P, kc_n, P], f16, tag="xt")
                h = kc_n // 2
                nc.sync.dma_start(out=xt[:, 0:h, :], in_=x_ext[kqi, db, :, 0:h, :])
                nc.sync.dma_start(out=xt[:, h:kc_n, :], in_=x_ext[kqi, db, :, h:kc_n, :])

                cprev = None
                if kqi > 0:
                    cprev = cprev_pool.tile([P, ncols], f16, tag="cprev")
                    hw = ncols // 2
                    nc.sync.dma_start(out=cprev[:, 0:hw], in_=c_accum[db, :, 0:hw])
                    nc.sync.dma_start(
                        out=cprev[:, hw:ncols], in_=c_accum[db, :, hw:ncols]
                    )

                accs = [
                    acc_psum.tile([P, nbw], f32, tag="acc", name=f"acc{i}")
                    for i in range(nnb)
                ]
                for kc in range(kc_n):
                    for nb in range(nnb):
                        nc.tensor.matmul(
                            accs[nb][:],
                            xt[:, kc, :],
                            panel[:, kc, nb * nbw : (nb + 1) * nbw],
                            start=(kc == 0),
                            stop=(kc == kc_n - 1),
                        )

                hw = ncols // 2
                if kqi < kq - 1:
                    stag = stag_pool.tile([P, ncols], f16, tag="stag")
                    for nb in range(nnb):
                        sl = slice(nb * nbw, (nb + 1) * nbw)
                        if kqi == 0:
                            nc.vector.tensor_copy(out=stag[:, sl], in_=accs[nb][:])
                        else:
                            nc.vector.tensor_tensor(
                                out=stag[:, sl],
                                in0=accs[nb][:],
                                in1=cprev[:, sl],
                                op=mybir.AluOpType.add,
                            )
                    nc.sync.dma_start(out=c_accum[db, :, 0:hw], in_=stag[:, 0:hw])
                    nc.sync.dma_start(
                        out=c_accum[db, :, hw:ncols], in_=stag[:, hw:ncols]
                    )
                else:
                    osb = out_pool.tile([P, ncols], f32, tag="osb")
                    for nb in range(nnb):
                        sl = slice(nb * nbw, (nb + 1) * nbw)
                        nc.vector.tensor_tensor(
                            out=osb[:, sl],
                            in0=accs[nb][:],
                            in1=cprev[:, sl],
                            op=mybir.AluOpType.add,
                        )
                    for nb in range(nnb):
                        sl = slice(nb * nbw, (nb + 1) * nbw)
                        nc.sync.dma_start(
                            out=out_ext[db * P : (db + 1) * P, sl], in_=osb[:, sl]
                        )

    nc.compile()
    return nc


_NC_CACHE = {}


def _get_program(**kw):
    key = tuple(sorted(kw.items()))
    if key not in _NC_CACHE:
        _NC_CACHE[key] = build_program(**kw)
    return _NC_CACHE[key]


def prepare_in_maps(x, adj, kq=KQ):
    """Host-side shard + transpose + fp16 cast. Returns in_maps for 8 cores."""
    from concurrent.futures import ThreadPoolExecutor

    f16 = np.float16
    kc_n = (N // kq) // P
    ndb = D // P

    def tile_x(b):
        xb = x[b].astype(f16)  # [D, M]
        t = xb.reshape(ndb, P, kq, kc_n, P)  # (db, dd, kqi, kc, p)
        return np.ascontiguousarray(t.transpose(2, 0, 4, 3, 1))

    def make_at(c):
        b, nq = divmod(c, NSPLIT)
        asl = adj[b, nq * NC : (nq + 1) * NC, :].astype(f16)  # [NC, M]
        return np.ascontiguousarray(asl.T)

    with ThreadPoolExecutor(max_workers=10) as ex:
        xt_f = {b: ex.submit(tile_x, b) for b in range(B)}
        at_f = {c: ex.submit(make_at, c) for c in range(NCORES)}
        in_maps = []
        for c in range(NCORES):
            b, _ = divmod(c, NSPLIT)
            in_maps.append({"x": xt_f[b].result(), "adj": at_f[c].result()})
    return in_maps


def assemble_output(results):
    out = np.empty((B, D, N), dtype=np.float32)
    for c in range(NCORES):
        b, nq = divmod(c, NSPLIT)
        out[b, :, nq * NC : (nq + 1) * NC] = results[c]["out"]
    return out


def kernel(x: np.ndarray, adj: np.ndarray) -> np.ndarray:
    """Full inputs in, full output out. x [B,D,N] f32, adj [B,N,N] f32."""
    from concourse.bass_utils import run_bass_kernel_spmd

    assert x.shape == (B, D, N) and adj.shape == (B, N, N)
    nc = _get_program()
    in_maps = prepare_in_maps(x, adj)
    res = run_bass_kernel_spmd(nc, in_maps, core_ids=list(range(NCORES)))
    return assemble_output(res.results)


# revision 9
# speedup vs baseline: 1.0135x; 1.0135x over previous
"""Bass/Trainium2 kernel for batched GNN message passing:
    out[b, d, n] = sum_m adj[b, n, m] * x[b, d, m]
B=2, D=3072, N=8192, fp32.

Sharding: 8 cores, core c -> (b = c//4, n-quarter = c%4). Each core computes
C[d, n_quarter] = X[b] @ A[b, n_quarter, :].T  with D=3072, NC=2048, M=8192.
Zero collectives; host slices inputs and concatenates outputs.

v4: fp16 operands, host-side transpose (no on-device transposes). Uneven
K-passes (4,12,16,16,16 chunks of 128) so the first adjT panel is small and
matmuls start ~16us in; later panels prefetch on the Activation sequencer
while the previous pass computes. Back-to-back fp16 matmuls (1 cyc/row) into
4 PSUM banks per d-block, alternating bank sets. Partials round-trip DRAM in
fp16; final output is written fp16 and upcast on the host.
"""

import sys
from contextlib import ExitStack

import numpy as np

sys.path.insert(0, "/opt/trn_rl_repo")

B = 2
D = 3072
N = 8192
NCORES = 8
NSPLIT = 4  # n-quarters per batch sample
NC = N // NSPLIT  # 2048 columns of out per core
P = 128
KC_TOTAL = N // P  # 64 contraction chunks of 128
KSPLITS = (4, 12, 16, 16, 16)  # kc chunks per K-pass


def build_program(d=D, ncols=NC, m=N, ksplits=KSPLITS, nbw=512):
    """Build the per-core Bass program. Returns compiled nc."""
    import concourse.mybir as mybir
    import concourse.tile as tile
    from concourse import bacc

    f32 = mybir.dt.float32
    f16 = mybir.dt.float16

    kc_total = m // P
    assert sum(ksplits) == kc_total
    ndb = d // P          # d-blocks
    nnb = ncols // nbw    # psum banks swept per d-block
    kq = len(ksplits)

    nc = bacc.Bacc(None, target_bir_lowering=False, debug=False)

    # xT tiled on host: x_ext[db, p, g, dd] = x[db*P+dd, g*P+p]
    x_ext = nc.dram_tensor("x", [ndb, P, kc_total, P], f16, kind="ExternalInput")
    # adjT plain: a_ext[m, n]
    a_ext = nc.dram_tensor("adj", [m, ncols], f16, kind="ExternalInput")
    out_ext = nc.dram_tensor("out", [d, ncols], f16, kind="ExternalOutput")

    with tile.TileContext(nc) as tc, ExitStack() as ctx:
        dram = ctx.enter_context(tc.tile_pool(name="dram", bufs=1, space="DRAM"))
        c_accum = dram.tile([ndb, P, ncols], f16, name="c_accum")

        panel_pool = ctx.enter_context(tc.tile_pool(name="panel", bufs=2))
        xt_pool = ctx.enter_context(tc.tile_pool(name="xt", bufs=4))
        stag_pool = ctx.enter_context(tc.tile_pool(name="stag", bufs=3))
        out_pool = ctx.enter_context(tc.tile_pool(name="outp", bufs=3))
        cprev_pool = ctx.enter_context(tc.tile_pool(name="cprev", bufs=3))
        acc_psum = ctx.enter_context(tc.tile_pool(name="accp", bufs=8, space="PSUM"))

        def load_xt(kqi, db, off, kc_n):
            xt = xt_pool.tile([P, kc_n, P], f16, tag="xt")
            h = kc_n // 2
            nc.sync.dma_start(out=xt[:, 0:h, :], in_=x_ext[db, :, off : off + h, :])
            nc.sync.dma_start(
                out=xt[:, h:kc_n, :], in_=x_ext[db, :, off + h : off + kc_n, :]
            )
            return xt

        off = 0
        for kqi, kc_n in enumerate(ksplits):
            mlo = off * P

            xt_pre = {}
            if kqi == 0:
                # Issue the first d-blocks' x strips before the panel chunks so
                # they land in time for the earliest matmuls.
                for db in (0, 1):
                    xt_pre[db] = load_xt(kqi, db, off, kc_n)

            # ---- load adjT panel [P, kc_n, ncols] for this K-pass ----
            panel = panel_pool.tile([P, kc_n, ncols], f16, tag="panel")
            if kqi == 0:
                # Small first panel, split column-wise across both HWDGE
                # engines so it lands (on many DMA queues) as fast as possible.
                csz = ncols // 2
                for kc in range(kc_n):
                    for q, eng in enumerate((nc.sync, nc.scalar)):
                        eng.dma_start(
                            out=panel[:, kc, q * csz : (q + 1) * csz],
                            in_=a_ext[
                                mlo + kc * P : mlo + (kc + 1) * P,
                                q * csz : (q + 1) * csz,
                            ],
                        )
            else:
                # Panel for pass 1 must finish while pass 0 (short) computes:
                # issue it on the otherwise-idle Activation sequencer. Later
                # panels have hundreds of us of slack; SP is fine.
                eng = nc.scalar if kqi == 1 else nc.sync
                for kc in range(kc_n):
                    eng.dma_start(
                        out=panel[:, kc, :],
                        in_=a_ext[mlo + kc * P : mlo + (kc + 1) * P, :],
                    )

            # ---- d-block loop ----
            for db in range(ndb):
                xt = xt_pre.pop(db, None)
                if xt is None:
                    xt = load_xt(kqi, db, off, kc_n)

                cprev = None
                if kqi > 0:
                    cprev = cprev_pool.tile([P, ncols], f16, tag="cprev")
                    hw = ncols // 2
                    nc.sync.dma_start(out=cprev[:, 0:hw], in_=c_accum[db, :, 0:hw])
                    nc.sync.dma_start(
                        out=cprev[:, hw:ncols], in_=c_accum[db, :, hw:ncols]
                    )

                accs = [
                    acc_psum.tile([P, nbw], f32, tag="acc", name=f"acc{i}")
                    for i in range(nnb)
                ]
                for kc in range(kc_n):
                    for nb in range(nnb):
                        nc.tensor.matmul(
                            accs[nb][:],
                            xt[:, kc, :],
                            panel[:, kc, nb * nbw : (nb + 1) * nbw],
                            start=(kc == 0),
                            stop=(kc == kc_n - 1),
                        )

                if kqi < kq - 1:
                    stag = stag_pool.tile([P, ncols], f16, tag="stag")
                    for nb in range(nnb):
                        sl = slice(nb * nbw, (nb + 1) * nbw)
                        if kqi == 0:
                            nc.vector.tensor_copy(out=stag[:, sl], in_=accs[nb][:])
                        else:
                            nc.vector.tensor_tensor(
                                out=stag[:, sl],
                                in0=accs[nb][:],
                                in1=cprev[:, sl],
                                op=mybir.AluOpType.add,
                            )
                    hw = ncols // 2
                    nc.sync.dma_start(out=c_accum[db, :, 0:hw], in_=stag[:, 0:hw])
                    nc.sync.dma_start(
                        out=c_accum[db, :, hw:ncols], in_=stag[:, hw:ncols]
                    )
                else:
                    osb = out_pool.tile([P, ncols], f16, tag="osb")
                    for nb in range(nnb):
                        sl = slice(nb * nbw, (nb + 1) * nbw)
                        nc.vector.tensor_tensor(
                            out=osb[:, sl],
                            in0=accs[nb][:],
                            in1=cprev[:, sl],
                            op=mybir.AluOpType.add,
                        )
                    for nb in range(nnb):
                        sl = slice(nb * nbw, (nb + 1) * nbw)
                        nc.sync.dma_start(
                            out=out_ext[db * P : (db + 1) * P, sl], in_=osb[:, sl]
                        )
            off += kc_n

    nc.compile()
    return nc


_NC_CACHE = {}


def _get_program(**kw):
    key = tuple(sorted(kw.items()))
    if key not in _NC_CACHE:
        _NC_CACHE[key] = build_program(**kw)
    return _NC_CACHE[key]


def prepare_in_maps(x, adj):
    """Host-side shard + transpose + fp16 cast. Returns in_maps for 8 cores."""
    from concurrent.futures import ThreadPoolExecutor

    f16 = np.float16
    ndb = D // P

    def tile_x(b):
        xb = x[b].astype(f16)  # [D, M]
        t = xb.reshape(ndb, P, KC_TOTAL, P)  # (db, dd, g, p)
        return np.ascontiguousarray(t.transpose(0, 3, 2, 1))  # [db, p, g, dd]

    def make_at(c):
        b, nq = divmod(c, NSPLIT)
        asl = adj[b, nq * NC : (nq + 1) * NC, :].astype(f16)  # [NC, M]
        return np.ascontiguousarray(asl.T)

    with ThreadPoolExecutor(max_workers=10) as ex:
        xt_f = {b: ex.submit(tile_x, b) for b in range(B)}
        at_f = {c: ex.submit(make_at, c) for c in range(NCORES)}
        in_maps = []
        for c in range(NCORES):
            b, _ = divmod(c, NSPLIT)
            in_maps.append({"x": xt_f[b].result(), "adj": at_f[c].result()})
    return in_maps


def assemble_output(results):
    from concurrent.futures import ThreadPoolExecutor

    out = np.empty((B, D, N), dtype=np.float32)

    def put(c):
        b, nq = divmod(c, NSPLIT)
        out[b, :, nq * NC : (nq + 1) * NC] = results[c]["out"]

    with ThreadPoolExecutor(max_workers=8) as ex:
        list(ex.map(put, range(NCORES)))
    return out


def kernel(x: np.ndarray, adj: np.ndarray) -> np.ndarray:
    """Full inputs in, full output out. x [B,D,N] f32, adj [B,N,N] f32."""
    from concourse.bass_utils import run_bass_kernel_spmd

    assert x.shape == (B, D, N) and adj.shape == (B, N, N)
    nc = _get_program()
    in_maps = prepare_in_maps(x, adj)
    res = run_bass_kernel_spmd(nc, in_maps, core_ids=list(range(NCORES)))
    return assemble_output(res.results)
